# revision 10
# baseline (speedup 1.0000x reference)
"""Trainium2 Bass kernel for nn_MirrorSystem (vq_codebook soft-VQ).

Strategy (data-parallel over the batch axis, B=8 -> 8 NeuronCores):
  Each core handles one batch element b in a fully transposed layout
  [feature/code, token] so that no on-device transposes are needed:

    zcT  [n, s] = codebook   @ z_flat^T      (M1, bf16)
    gbT  [m, s] = adjacency^T@ prev^T        (M2, bf16; lhsT = adjacency as given)
    d_total^T   = (|z|^2 + |c|^2 - 2 zcT)/2D - 0.1*softplus(var)*sigmoid(gbT)
    E = exp(-d_total^T);  z_q^T = (codebook^T @ E) / colsum(E)   (M3, fp32r)

  Cross-partition reductions (per-token |z|^2, softmax denominator) are done
  with ones-matrix matmuls which replicate the row-vector result across all
  128 partitions, so plain elementwise multiplies apply them.

  The global batch-variance scalar (ComplexityModulator) is computed on
  device: per-core partial sums of |z| and |z|^2 -> tiny [128,2] AllReduce
  across the 8 cores -> softplus chain on device -> per-partition scalar.
  Work not needing that scalar (M1/M2/dcon/sigmoid) runs concurrently with
  the statistics phase; only the final exp/softmax tail waits for it.

  The scalar energy mean is shipped out as per-core partial sums and
  combined on host during unsharding.
"""

import numpy as np
import ml_dtypes

import concourse.mybir as mybir
import concourse.tile as tile
from concourse import bacc
from concourse.bass_utils import run_bass_kernel_spmd

F32 = mybir.dt.float32
F32R = mybir.dt.float32r
BF16 = mybir.dt.bfloat16
AF = mybir.ActivationFunctionType
OP = mybir.AluOpType
AX = mybir.AxisListType

# problem shape (hardcoded per contest rules)
B, S, D, N = 8, 4096, 512, 1024
D2 = 2 * D                       # 1024 concat features
NCORES = 8
KC = D2 // 128                   # 8 feature chunks
NT = N // 128                    # 8 code tiles
SC = 8                           # s-chunks per core
SW = S // SC                     # 512 tokens per chunk
LAMBDA_ENERGY = 0.1
EPS = 1e-6
MTOT = float(B * S * D)          # elements in the variance reduction

_CACHE = {}


class _PinActTable:
    """During bacc compile, present all activation-table sets as empty except
    natural_log_exp_and_others (ids preserved), so every activation lands in
    the one table that covers Square/Ln/Exp/Identity/Copy and the kernel pays
    a single table load instead of one per Ln<->Exp alternation."""

    KEEP = "natural_log_exp_and_others"

    def __enter__(self):
        import concourse.bacc as bacc_mod
        import concourse.hw_specs as hw_specs

        self._mod = bacc_mod
        self._orig = bacc_mod.get_activation_tables
        full = hw_specs.get_activation_tables("gen3")
        assert self.KEEP in full
        patched = {k: (v if k == self.KEEP else set()) for k, v in full.items()}
        bacc_mod.get_activation_tables = lambda arch: patched
        return self

    def __exit__(self, *exc):
        self._mod.get_activation_tables = self._orig
        return False


def build_nc():
    nc = bacc.Bacc("TRN2", target_bir_lowering=False, debug=False,
                   num_devices=NCORES)

    zt_d = nc.dram_tensor("zt", [SC, KC, 128, SW], BF16, kind="ExternalInput").ap()
    pt_d = nc.dram_tensor("pt", [SC, NT, 128, SW], BF16, kind="ExternalInput").ap()
    cbt_d = nc.dram_tensor("cbt", [KC, 128, N], BF16, kind="ExternalInput").ap()
    adj_d = nc.dram_tensor("adj", [NT, 128, N], BF16, kind="ExternalInput").ap()
    cb_d = nc.dram_tensor("cb", [NT, 128, D2], F32, kind="ExternalInput").ap()

    zq_d = nc.dram_tensor("zq", [KC, 128, S], F32, kind="ExternalOutput").ap()
    eneg_d = nc.dram_tensor("eneg", [128, SC * NT], F32, kind="ExternalOutput").ap()
    misc_d = nc.dram_tensor("misc", [128, 4], F32, kind="ExternalOutput").ap()

    with tile.TileContext(nc) as tc:
        with tc.tile_pool(name="wq", bufs=1) as wq, \
             tc.tile_pool(name="sb", bufs=1) as sb, \
             tc.tile_pool(name="ps", bufs=1, space="PSUM") as ps, \
             tc.tile_pool(name="dram", bufs=1, space="DRAM") as dram:

            # ---- resident weights (sync DMA queue) ---------------------
            cbt_t = [wq.tile([128, N], BF16, name=f"cbt{c}") for c in range(KC)]
            adj_t = [wq.tile([128, N], BF16, name=f"adj{c}") for c in range(NT)]
            for c in range(KC):
                nc.sync.dma_start(out=cbt_t[c], in_=cbt_d[c])
            for c in range(NT):
                nc.sync.dma_start(out=adj_t[c], in_=adj_d[c])

            # fp32 codebook streamed through shared slots; rounded copy kept
            cb_r = [wq.tile([128, D2], F32R, name=f"cbr{c}") for c in range(NT)]
            cnorm = sb.tile([128, NT], F32, tag="cnorm")
            for c in range(NT):
                cb_f = sb.tile([128, D2], F32, tag="mag2", bufs=3, name=f"cbf{c}")
                nc.sync.dma_start(out=cb_f, in_=cb_d[c])
                nc.vector.tensor_copy(cb_r[c], cb_f)
                sq_scr = sb.tile([128, D2], F32, tag="lnm", bufs=2,
                                 name=f"sqscr{c}")
                nc.scalar.activation(sq_scr, cb_f, AF.Square,
                                     accum_out=cnorm[:, c:c + 1])
            cnorm_sc = sb.tile([128, NT], F32, tag="cnorm_sc")
            nc.vector.tensor_scalar_mul(cnorm_sc, cnorm, 1.0 / D2)

            # ---- constants ---------------------------------------------
            ones_f = sb.tile([128, 128], F32, tag="ones_f")
            nc.vector.memset(ones_f, 1.0)
            ones_r = sb.tile([128, 128], F32R, tag="ones_r")
            nc.vector.tensor_copy(ones_r, ones_f)
            oos_f = sb.tile([128, 128], F32, tag="oos_f")
            nc.vector.memset(oos_f, 1.0 / D2)
            oos_r = sb.tile([128, 128], F32R, tag="oos_r")
            nc.vector.tensor_copy(oos_r, oos_f)

            # ---- per-chunk: phase A stats + phase B prework -------------
            corder = [0, 4, 1, 5, 2, 6, 3, 7]
            znorm_sc = [None] * SC
            s1cols = sb.tile([128, SC * 4], F32, tag="s1cols")
            s2cols = sb.tile([128, SC * 4], F32, tag="s2cols")
            dcon_t = [[None] * NT for _ in range(SC)]
            sig_t = [[None] * NT for _ in range(SC)]
            zt_t = [[None] * KC for _ in range(SC)]
            pt_t = [[None] * NT for _ in range(SC)]

            for j in range(SC):
                # --- phase A: stats over z (vector DMA queue for zta) ---
                zsq = [None] * KC
                znorm_ps = ps.tile([128, SW], F32, tag="znorm_ps", bufs=2,
                                   name=f"znps{j}")
                for ci, c in enumerate(corder):
                    t = sb.tile([128, SW], BF16, tag="zta", bufs=5,
                                name=f"zta{j}_{c}")
                    nc.gpsimd.dma_start(out=t, in_=zt_d[j, c])
                    zq_sq = sb.tile([128, SW], F32R, tag="zsq", bufs=5,
                                    name=f"zsq{j}_{c}")
                    nc.vector.tensor_mul(zq_sq, t, t)
                    zsq[c] = zq_sq
                    nc.tensor.matmul(znorm_ps, oos_r, zq_sq,
                                     start=(ci == 0), stop=(ci == KC - 1))
                    if c >= 4:
                        cp = c - 4
                        mag2 = sb.tile([128, SW], F32, tag="mag2", bufs=3,
                                       name=f"mag2_{j}_{cp}")
                        nc.vector.scalar_tensor_tensor(
                            mag2, zsq[cp].bitcast(F32), 1.0,
                            zq_sq.bitcast(F32), op0=OP.mult, op1=OP.add,
                            accum_out=s2cols[:, j * 4 + cp:j * 4 + cp + 1])
                        lnm = sb.tile([128, SW], F32, tag="lnm", bufs=2,
                                      name=f"lnm_{j}_{cp}")
                        nc.scalar.activation(lnm, mag2, AF.Ln)
                        mag = sb.tile([128, SW], F32, tag="mag", bufs=2,
                                      name=f"mag_{j}_{cp}")
                        nc.scalar.activation(
                            mag, lnm, AF.Exp, scale=0.5,
                            accum_out=s1cols[:, j * 4 + cp:j * 4 + cp + 1])
                zn = sb.tile([128, SW], F32, tag="znorm_sc", bufs=SC,
                             name=f"znorm{j}")
                nc.scalar.copy(zn, znorm_ps)
                znorm_sc[j] = zn

                # --- phase B prework: loads + M1/M2 + dcon + sigmoid ----
                for c in range(KC):
                    t = sb.tile([128, SW], BF16, tag="zt", bufs=10,
                                name=f"zt{j}_{c}")
                    nc.sync.dma_start(out=t, in_=zt_d[j, c])
                    zt_t[j][c] = t
                for c in range(NT):
                    t = sb.tile([128, SW], BF16, tag="pt", bufs=10,
                                name=f"pt{j}_{c}")
                    nc.sync.dma_start(out=t, in_=pt_d[j, c])
                    pt_t[j][c] = t
                for i in range(NT):
                    zc_ps = ps.tile([128, SW], F32, tag="zc_ps", bufs=2,
                                    name=f"zc{j}_{i}")
                    for ci in range(KC):
                        nc.tensor.matmul(
                            zc_ps, cbt_t[ci][:, i * 128:(i + 1) * 128],
                            zt_t[j][ci], start=(ci == 0), stop=(ci == KC - 1))
                    dcon = sb.tile([128, SW], F32, tag="dcon", bufs=8,
                                   name=f"dcon{j}_{i}")
                    nc.scalar.activation(dcon, zc_ps, AF.Identity,
                                         bias=cnorm_sc[:, i:i + 1],
                                         scale=-2.0 / D2)
                    nc.vector.tensor_add(dcon, dcon, znorm_sc[j])
                    dcon_t[j][i] = dcon

                    gb_ps = ps.tile([128, SW], F32, tag="gb_ps", bufs=2,
                                    name=f"gb{j}_{i}")
                    for ci in range(NT):
                        nc.tensor.matmul(
                            gb_ps, adj_t[ci][:, i * 128:(i + 1) * 128],
                            pt_t[j][ci], start=(ci == 0), stop=(ci == NT - 1))
                    sig = sb.tile([128, SW], F32, tag="sig", bufs=8,
                                  name=f"sig{j}_{i}")
                    nc.scalar.activation(sig, gb_ps, AF.Exp, scale=-1.0)
                    nc.gpsimd.tensor_scalar_add(sig, sig, 1.0)
                    nc.vector.reciprocal_approx_fast(out=sig, in_=sig)
                    sig_t[j][i] = sig

            # ---- collective: global |z| stats -> modulation scalar ------
            s12 = sb.tile([128, 2], F32, tag="s12")
            nc.vector.tensor_reduce(s12[:, 0:1], s1cols, axis=AX.X, op=OP.add)
            nc.vector.tensor_reduce(s12[:, 1:2], s2cols, axis=AX.X, op=OP.add)
            cc_in = dram.tile([128, 2], F32, name="cc_in")
            cc_out = dram.tile([128, 2], F32, name="cc_out", addr_space="Shared")
            nc.gpsimd.dma_start(out=cc_in, in_=s12)
            nc.gpsimd.collective_compute(
                "AllReduce", OP.add,
                replica_groups=[list(range(NCORES))],
                ins=[cc_in.opt()], outs=[cc_out.opt()])
            red = sb.tile([128, 2], F32, tag="red")
            nc.gpsimd.dma_start(out=red, in_=cc_out)
            rep_ps = ps.tile([128, 2], F32, tag="znorm_ps", bufs=2,
                             name="rep_ps")
            nc.tensor.matmul(rep_ps, ones_f, red, start=True, stop=True)

            # modulation chain, all [128,1] (identical values on every lane)
            ka = 1.0 / ((MTOT - 1.0) * (1.0 + EPS))
            kb = -1.0 / (MTOT * (MTOT - 1.0) * (1.0 + EPS))
            rep_sb = sb.tile([128, 2], F32, tag="rep_sb")
            nc.scalar.copy(rep_sb, rep_ps)
            s1sq = sb.tile([128, 1], F32, tag="s1sq")
            nc.vector.tensor_mul(s1sq, rep_sb[:, 0:1], rep_sb[:, 0:1])
            w2 = sb.tile([128, 1], F32, tag="w2")
            nc.scalar.activation(w2, rep_sb[:, 1:2], AF.Identity, scale=ka)
            var_t = sb.tile([128, 1], F32, tag="var_t")
            nc.scalar.activation(var_t, s1sq, AF.Identity, scale=kb, bias=w2)
            ev = sb.tile([128, 1], F32, tag="ev")
            nc.scalar.activation(ev, var_t, AF.Exp)
            modp = sb.tile([128, 1], F32, tag="modp")
            nc.scalar.activation(modp, ev, AF.Ln, bias=1.0)
            mod01 = sb.tile([128, 1], F32, tag="mod01")
            nc.scalar.activation(mod01, modp, AF.Identity, scale=LAMBDA_ENERGY)
            misc = sb.tile([128, 4], F32, tag="misc")
            nc.vector.tensor_copy(misc[:, 0:1], mod01)
            nc.vector.tensor_copy(misc[:, 1:2], rep_sb[:, 0:1])
            nc.vector.tensor_copy(misc[:, 2:3], rep_sb[:, 1:2])
            nc.vector.tensor_copy(misc[:, 3:4], var_t)
            nc.gpsimd.dma_start(out=misc_d, in_=misc)

            # ---- post-collective tail: softmax + output matmul ----------
            eneg = sb.tile([128, SC * NT], F32, tag="eneg")
            for j in range(SC):
                e_r = [None] * NT
                for i in range(NT):
                    neg_p = sb.tile([128, SW], F32, tag="neg_p", bufs=3,
                                    name=f"neg{j}_{i}")
                    col = j * NT + i
                    nc.vector.scalar_tensor_tensor(
                        neg_p, sig_t[j][i], mod01, dcon_t[j][i],
                        op0=OP.mult, op1=OP.subtract,
                        accum_out=eneg[:, col:col + 1])
                    er = sb.tile([128, SW], F32R, tag="e_r", bufs=8,
                                 name=f"er{j}_{i}")
                    nc.scalar.activation(er, neg_p, AF.Exp)
                    e_r[i] = er

                se_ps = ps.tile([128, SW], F32, tag="znorm_ps", bufs=2,
                                name=f"se{j}")
                for i in range(NT):
                    nc.tensor.matmul(se_ps, ones_r, e_r[i],
                                     start=(i == 0), stop=(i == NT - 1))
                rcp = sb.tile([128, SW], F32, tag="rcp", bufs=2,
                              name=f"rcp{j}")
                nc.vector.reciprocal_approx_fast(out=rcp, in_=se_ps)

                for k in range(KC):
                    zq_ps = ps.tile([128, SW], F32, tag="zq_ps", bufs=2,
                                    name=f"zqps{j}_{k}")
                    for i in range(NT):
                        nc.tensor.matmul(
                            zq_ps, cb_r[i][:, k * 128:(k + 1) * 128], e_r[i],
                            start=(i == 0), stop=(i == NT - 1))
                    zq_sb = sb.tile([128, SW], F32, tag="zq_sb", bufs=3,
                                    name=f"zqsb{j}_{k}")
                    nc.vector.tensor_mul(zq_sb, zq_ps, rcp)
                    nc.scalar.dma_start(out=zq_d[k][:, j * SW:(j + 1) * SW],
                                        in_=zq_sb)

            nc.sync.dma_start(out=eneg_d, in_=eneg)

    with _PinActTable():
        nc.compile()
    return nc


def prep_inputs(z_real, z_imag, prev_sym_dist, codebook, adjacency_energy):
    bf16 = ml_dtypes.bfloat16
    cbt = np.ascontiguousarray(np.asarray(codebook).T).astype(bf16).reshape(KC, 128, N)
    adj = np.ascontiguousarray(np.asarray(adjacency_energy)).astype(bf16).reshape(NT, 128, N)
    cb = np.ascontiguousarray(np.asarray(codebook), dtype=np.float32).reshape(NT, 128, D2)

    in_maps = []
    for b in range(B):
        zT = np.empty((D2, S), dtype=bf16)
        zT[:D] = np.asarray(z_real[b]).T
        zT[D:] = np.asarray(z_imag[b]).T
        pT = np.asarray(prev_sym_dist[b]).T.astype(bf16)
        # [D2, S] -> [SC, KC, 128, SW]
        zt = np.ascontiguousarray(
            zT.reshape(KC, 128, SC, SW).transpose(2, 0, 1, 3))
        pt = np.ascontiguousarray(
            np.asarray(pT).reshape(NT, 128, SC, SW).transpose(2, 0, 1, 3))
        in_maps.append({"zt": zt, "pt": pt, "cbt": cbt, "adj": adj, "cb": cb})
    return in_maps


def postprocess(results):
    zq_real = np.empty((B, S, D), dtype=np.float32)
    zq_imag = np.empty((B, S, D), dtype=np.float32)
    eneg_total = 0.0
    for b in range(B):
        r = results[b]
        zqT = r["zq"].reshape(D2, S)          # [feature, token]
        zq = zqT.T                            # [token, feature]
        zq_real[b] = zq[:, :D]
        zq_imag[b] = zq[:, D:]
        eneg_total += r["eneg"].sum(dtype=np.float64)
    energy = np.float32(-eneg_total / (B * S * N))
    return zq_real, zq_imag, energy


def kernel(z_real, z_imag, prev_sym_dist, codebook, adjacency_energy):
    if "nc" not in _CACHE:
        _CACHE["nc"] = build_nc()
    nc = _CACHE["nc"]
    in_maps = prep_inputs(z_real, z_imag, prev_sym_dist, codebook,
                          adjacency_energy)
    res = run_bass_kernel_spmd(nc, in_maps, list(range(NCORES)))
    return postprocess(res.results)


# revision 11
# speedup vs baseline: 1.4385x; 1.4385x over previous
"""Trainium2 Bass kernel for nn_MirrorSystem (vq_codebook soft-VQ).

Strategy (data-parallel over the batch axis, B=8 -> 8 NeuronCores):
  Each core handles one batch element b in a fully transposed layout
  [feature/code, token] so that no on-device transposes are needed:

    zcT  [n, s] = codebook   @ z_flat^T      (M1, bf16)
    gbT  [m, s] = adjacency^T@ prev^T        (M2, bf16; lhsT = adjacency as given)
    d_total^T   = (|z|^2 + |c|^2 - 2 zcT)/2D - 0.1*softplus(var)*sigmoid(gbT)
    E = exp(-d_total^T);  z_q^T = (codebook^T @ E) / colsum(E)   (M3, fp32r)

  Cross-partition reductions (per-token |z|^2, softmax denominator) are done
  with ones-matrix matmuls which replicate the row-vector result across all
  128 partitions, so plain elementwise multiplies apply them.

  The global batch-variance scalar (ComplexityModulator) is computed on
  device: per-core partial sums of |z| and |z|^2 -> tiny [128,2] AllReduce
  across the 8 cores -> softplus chain on device -> per-partition scalar.
  Work not needing that scalar (M1/M2/dcon/sigmoid) runs concurrently with
  the statistics phase; only the final exp/softmax tail waits for it.

  The scalar energy mean is shipped out as per-core partial sums and
  combined on host during unsharding.
"""

import numpy as np
import ml_dtypes

import concourse.mybir as mybir
import concourse.tile as tile
from concourse import bacc
from concourse.bass_utils import run_bass_kernel_spmd

F32 = mybir.dt.float32
F32R = mybir.dt.float32r
BF16 = mybir.dt.bfloat16
AF = mybir.ActivationFunctionType
OP = mybir.AluOpType
AX = mybir.AxisListType

# problem shape (hardcoded per contest rules)
B, S, D, N = 8, 4096, 512, 1024
D2 = 2 * D                       # 1024 concat features
NCORES = 8
KC = D2 // 128                   # 8 feature chunks
NT = N // 128                    # 8 code tiles
SC = 8                           # s-chunks per core
SW = S // SC                     # 512 tokens per chunk
LAMBDA_ENERGY = 0.1
EPS = 1e-6
MTOT = float(B * S * D)          # elements in the variance reduction

_CACHE = {}


class _PinActTable:
    """During bacc compile, present all activation-table sets as empty except
    natural_log_exp_and_others (ids preserved), so every activation lands in
    the one table that covers Square/Ln/Exp/Identity/Copy and the kernel pays
    a single table load instead of one per Ln<->Exp alternation."""

    KEEP = "natural_log_exp_and_others"

    def __enter__(self):
        import concourse.bacc as bacc_mod
        import concourse.hw_specs as hw_specs

        self._mod = bacc_mod
        self._orig = bacc_mod.get_activation_tables
        full = hw_specs.get_activation_tables("gen3")
        assert self.KEEP in full
        patched = {k: (v if k == self.KEEP else set()) for k, v in full.items()}
        bacc_mod.get_activation_tables = lambda arch: patched
        return self

    def __exit__(self, *exc):
        self._mod.get_activation_tables = self._orig
        return False


def build_nc():
    nc = bacc.Bacc("TRN2", target_bir_lowering=False, debug=False,
                   num_devices=NCORES)

    zt_d = nc.dram_tensor("zt", [SC, KC, 128, SW], BF16, kind="ExternalInput").ap()
    pt_d = nc.dram_tensor("pt", [SC, NT, 128, SW], BF16, kind="ExternalInput").ap()
    cbt_d = nc.dram_tensor("cbt", [KC, 128, N], BF16, kind="ExternalInput").ap()
    adj_d = nc.dram_tensor("adj", [NT, 128, N], BF16, kind="ExternalInput").ap()
    cb_d = nc.dram_tensor("cb", [NT, 128, D2], F32, kind="ExternalInput").ap()

    zq_d = nc.dram_tensor("zq", [KC, 128, S], F32, kind="ExternalOutput").ap()
    eneg_d = nc.dram_tensor("eneg", [128, SC * NT], F32, kind="ExternalOutput").ap()
    misc_d = nc.dram_tensor("misc", [128, 4], F32, kind="ExternalOutput").ap()

    with tile.TileContext(nc) as tc:
        with tc.tile_pool(name="wq", bufs=1) as wq, \
             tc.tile_pool(name="sb", bufs=1) as sb, \
             tc.tile_pool(name="ps", bufs=1, space="PSUM") as ps, \
             tc.tile_pool(name="dram", bufs=1, space="DRAM") as dram:

            # ---- resident weights (sync DMA queue) ---------------------
            cbt_t = [wq.tile([128, N], BF16, name=f"cbt{c}") for c in range(KC)]
            adj_t = [wq.tile([128, N], BF16, name=f"adj{c}") for c in range(NT)]
            for c in range(KC):
                nc.sync.dma_start(out=cbt_t[c], in_=cbt_d[c])
            for c in range(NT):
                nc.sync.dma_start(out=adj_t[c], in_=adj_d[c])

            # fp32 codebook streamed through shared slots; rounded copy kept
            cb_r = [wq.tile([128, D2], F32R, name=f"cbr{c}") for c in range(NT)]
            cnorm = sb.tile([128, NT], F32, tag="cnorm")
            for c in range(NT):
                cb_f = sb.tile([128, D2], F32, tag="mag2", bufs=3, name=f"cbf{c}")
                nc.sync.dma_start(out=cb_f, in_=cb_d[c])
                nc.vector.tensor_copy(cb_r[c], cb_f)
                sq_scr = sb.tile([128, D2], F32, tag="lnm", bufs=2,
                                 name=f"sqscr{c}")
                nc.scalar.activation(sq_scr, cb_f, AF.Square,
                                     accum_out=cnorm[:, c:c + 1])
            cnorm_sc = sb.tile([128, NT], F32, tag="cnorm_sc")
            nc.vector.tensor_scalar_mul(cnorm_sc, cnorm, 1.0 / D2)

            # ---- constants ---------------------------------------------
            ones_f = sb.tile([128, 128], F32, tag="ones_f")
            nc.vector.memset(ones_f, 1.0)
            ones_r = sb.tile([128, 128], F32R, tag="ones_r")
            nc.vector.tensor_copy(ones_r, ones_f)
            oos_f = sb.tile([128, 128], F32, tag="oos_f")
            nc.vector.memset(oos_f, 1.0 / D2)
            oos_r = sb.tile([128, 128], F32R, tag="oos_r")
            nc.vector.tensor_copy(oos_r, oos_f)

            # ---- per-chunk: phase A stats + phase B prework -------------
            corder = [0, 4, 1, 5, 2, 6, 3, 7]
            znorm_sc = [None] * SC
            s1cols = sb.tile([128, SC * 4], F32, tag="s1cols")
            s2cols = sb.tile([128, SC * 4], F32, tag="s2cols")
            dcon_t = [[None] * NT for _ in range(SC)]
            sig_t = [[None] * NT for _ in range(SC)]
            zt_t = [[None] * KC for _ in range(SC)]
            pt_t = [[None] * NT for _ in range(SC)]

            for j in range(SC):
                # --- phase A: stats over z (vector DMA queue for zta) ---
                zsq = [None] * KC
                znorm_ps = ps.tile([128, SW], F32, tag="znorm_ps", bufs=2,
                                   name=f"znps{j}")
                for ci, c in enumerate(corder):
                    t = sb.tile([128, SW], BF16, tag="zta", bufs=5,
                                name=f"zta{j}_{c}")
                    nc.gpsimd.dma_start(out=t, in_=zt_d[j, c])
                    zq_sq = sb.tile([128, SW], F32R, tag="zsq", bufs=5,
                                    name=f"zsq{j}_{c}")
                    nc.vector.tensor_mul(zq_sq, t, t)
                    zsq[c] = zq_sq
                    nc.tensor.matmul(znorm_ps, oos_r, zq_sq,
                                     start=(ci == 0), stop=(ci == KC - 1))
                    if c >= 4:
                        cp = c - 4
                        mag2 = sb.tile([128, SW], F32, tag="mag2", bufs=3,
                                       name=f"mag2_{j}_{cp}")
                        nc.vector.scalar_tensor_tensor(
                            mag2, zsq[cp].bitcast(F32), 1.0,
                            zq_sq.bitcast(F32), op0=OP.mult, op1=OP.add,
                            accum_out=s2cols[:, j * 4 + cp:j * 4 + cp + 1])
                        lnm = sb.tile([128, SW], F32, tag="lnm", bufs=2,
                                      name=f"lnm_{j}_{cp}")
                        nc.scalar.activation(lnm, mag2, AF.Ln)
                        mag = sb.tile([128, SW], F32, tag="mag", bufs=2,
                                      name=f"mag_{j}_{cp}")
                        nc.scalar.activation(
                            mag, lnm, AF.Exp, scale=0.5,
                            accum_out=s1cols[:, j * 4 + cp:j * 4 + cp + 1])
                zn = sb.tile([128, SW], F32, tag="znorm_sc", bufs=SC,
                             name=f"znorm{j}")
                nc.scalar.copy(zn, znorm_ps)
                znorm_sc[j] = zn

                # --- phase B prework: loads + M1/M2 + dcon + sigmoid ----
                for c in range(KC):
                    t = sb.tile([128, SW], BF16, tag="zt", bufs=10,
                                name=f"zt{j}_{c}")
                    nc.sync.dma_start(out=t, in_=zt_d[j, c])
                    zt_t[j][c] = t
                for c in range(NT):
                    t = sb.tile([128, SW], BF16, tag="pt", bufs=10,
                                name=f"pt{j}_{c}")
                    nc.sync.dma_start(out=t, in_=pt_d[j, c])
                    pt_t[j][c] = t
                for i in range(NT):
                    zc_ps = ps.tile([128, SW], F32, tag="zc_ps", bufs=2,
                                    name=f"zc{j}_{i}")
                    for ci in range(KC):
                        nc.tensor.matmul(
                            zc_ps, cbt_t[ci][:, i * 128:(i + 1) * 128],
                            zt_t[j][ci], start=(ci == 0), stop=(ci == KC - 1))
                    dcon = sb.tile([128, SW], F32, tag="dcon", bufs=8,
                                   name=f"dcon{j}_{i}")
                    nc.scalar.activation(dcon, zc_ps, AF.Identity,
                                         bias=cnorm_sc[:, i:i + 1],
                                         scale=-2.0 / D2)
                    nc.vector.tensor_add(dcon, dcon, znorm_sc[j])
                    dcon_t[j][i] = dcon

                    gb_ps = ps.tile([128, SW], F32, tag="gb_ps", bufs=2,
                                    name=f"gb{j}_{i}")
                    for ci in range(NT):
                        nc.tensor.matmul(
                            gb_ps, adj_t[ci][:, i * 128:(i + 1) * 128],
                            pt_t[j][ci], start=(ci == 0), stop=(ci == NT - 1))
                    sig = sb.tile([128, SW], F32, tag="sig", bufs=8,
                                  name=f"sig{j}_{i}")
                    nc.scalar.activation(sig, gb_ps, AF.Exp, scale=-1.0)
                    nc.vector.tensor_scalar_add(sig, sig, 1.0)
                    nc.vector.reciprocal_approx_fast(out=sig, in_=sig)
                    sig_t[j][i] = sig

            # ---- collective: global |z| stats -> modulation scalar ------
            s12 = sb.tile([128, 2], F32, tag="s12")
            nc.vector.tensor_reduce(s12[:, 0:1], s1cols, axis=AX.X, op=OP.add)
            nc.vector.tensor_reduce(s12[:, 1:2], s2cols, axis=AX.X, op=OP.add)
            cc_in = dram.tile([128, 2], F32, name="cc_in")
            cc_out = dram.tile([128, 2], F32, name="cc_out", addr_space="Shared")
            nc.gpsimd.dma_start(out=cc_in, in_=s12)
            nc.gpsimd.collective_compute(
                "AllReduce", OP.add,
                replica_groups=[list(range(NCORES))],
                ins=[cc_in.opt()], outs=[cc_out.opt()])
            red = sb.tile([128, 2], F32, tag="red")
            nc.gpsimd.dma_start(out=red, in_=cc_out)
            rep_ps = ps.tile([128, 2], F32, tag="znorm_ps", bufs=2,
                             name="rep_ps")
            nc.tensor.matmul(rep_ps, ones_f, red, start=True, stop=True)

            # modulation chain, all [128,1] (identical values on every lane)
            ka = 1.0 / ((MTOT - 1.0) * (1.0 + EPS))
            kb = -1.0 / (MTOT * (MTOT - 1.0) * (1.0 + EPS))
            rep_sb = sb.tile([128, 2], F32, tag="rep_sb")
            nc.scalar.copy(rep_sb, rep_ps)
            s1sq = sb.tile([128, 1], F32, tag="s1sq")
            nc.vector.tensor_mul(s1sq, rep_sb[:, 0:1], rep_sb[:, 0:1])
            w2 = sb.tile([128, 1], F32, tag="w2")
            nc.scalar.activation(w2, rep_sb[:, 1:2], AF.Identity, scale=ka)
            var_t = sb.tile([128, 1], F32, tag="var_t")
            nc.scalar.activation(var_t, s1sq, AF.Identity, scale=kb, bias=w2)
            ev = sb.tile([128, 1], F32, tag="ev")
            nc.scalar.activation(ev, var_t, AF.Exp)
            modp = sb.tile([128, 1], F32, tag="modp")
            nc.scalar.activation(modp, ev, AF.Ln, bias=1.0)
            mod01 = sb.tile([128, 1], F32, tag="mod01")
            nc.scalar.activation(mod01, modp, AF.Identity, scale=LAMBDA_ENERGY)
            misc = sb.tile([128, 4], F32, tag="misc")
            nc.vector.tensor_copy(misc[:, 0:1], mod01)
            nc.vector.tensor_copy(misc[:, 1:2], rep_sb[:, 0:1])
            nc.vector.tensor_copy(misc[:, 2:3], rep_sb[:, 1:2])
            nc.vector.tensor_copy(misc[:, 3:4], var_t)
            nc.gpsimd.dma_start(out=misc_d, in_=misc)

            # ---- post-collective tail: softmax + output matmul ----------
            eneg = sb.tile([128, SC * NT], F32, tag="eneg")
            for j in range(SC):
                e_r = [None] * NT
                for i in range(NT):
                    neg_p = sb.tile([128, SW], F32, tag="neg_p", bufs=3,
                                    name=f"neg{j}_{i}")
                    col = j * NT + i
                    nc.vector.scalar_tensor_tensor(
                        neg_p, sig_t[j][i], mod01, dcon_t[j][i],
                        op0=OP.mult, op1=OP.subtract,
                        accum_out=eneg[:, col:col + 1])
                    er = sb.tile([128, SW], F32R, tag="e_r", bufs=8,
                                 name=f"er{j}_{i}")
                    nc.scalar.activation(er, neg_p, AF.Exp)
                    e_r[i] = er

                se_ps = ps.tile([128, SW], F32, tag="znorm_ps", bufs=2,
                                name=f"se{j}")
                for i in range(NT):
                    nc.tensor.matmul(se_ps, ones_r, e_r[i],
                                     start=(i == 0), stop=(i == NT - 1))
                rcp = sb.tile([128, SW], F32, tag="rcp", bufs=2,
                              name=f"rcp{j}")
                nc.vector.reciprocal_approx_fast(out=rcp, in_=se_ps)

                for k in range(KC):
                    zq_ps = ps.tile([128, SW], F32, tag="zq_ps", bufs=2,
                                    name=f"zqps{j}_{k}")
                    for i in range(NT):
                        nc.tensor.matmul(
                            zq_ps, cb_r[i][:, k * 128:(k + 1) * 128], e_r[i],
                            start=(i == 0), stop=(i == NT - 1))
                    zq_sb = sb.tile([128, SW], F32, tag="zq_sb", bufs=3,
                                    name=f"zqsb{j}_{k}")
                    nc.vector.tensor_mul(zq_sb, zq_ps, rcp)
                    nc.scalar.dma_start(out=zq_d[k][:, j * SW:(j + 1) * SW],
                                        in_=zq_sb)

            nc.sync.dma_start(out=eneg_d, in_=eneg)

    with _PinActTable():
        nc.compile()
    return nc


def prep_inputs(z_real, z_imag, prev_sym_dist, codebook, adjacency_energy):
    bf16 = ml_dtypes.bfloat16
    cbt = np.ascontiguousarray(np.asarray(codebook).T).astype(bf16).reshape(KC, 128, N)
    adj = np.ascontiguousarray(np.asarray(adjacency_energy)).astype(bf16).reshape(NT, 128, N)
    cb = np.ascontiguousarray(np.asarray(codebook), dtype=np.float32).reshape(NT, 128, D2)

    in_maps = []
    for b in range(B):
        zT = np.empty((D2, S), dtype=bf16)
        zT[:D] = np.asarray(z_real[b]).T
        zT[D:] = np.asarray(z_imag[b]).T
        pT = np.asarray(prev_sym_dist[b]).T.astype(bf16)
        # [D2, S] -> [SC, KC, 128, SW]
        zt = np.ascontiguousarray(
            zT.reshape(KC, 128, SC, SW).transpose(2, 0, 1, 3))
        pt = np.ascontiguousarray(
            np.asarray(pT).reshape(NT, 128, SC, SW).transpose(2, 0, 1, 3))
        in_maps.append({"zt": zt, "pt": pt, "cbt": cbt, "adj": adj, "cb": cb})
    return in_maps


def postprocess(results):
    zq_real = np.empty((B, S, D), dtype=np.float32)
    zq_imag = np.empty((B, S, D), dtype=np.float32)
    eneg_total = 0.0
    for b in range(B):
        r = results[b]
        zqT = r["zq"].reshape(D2, S)          # [feature, token]
        zq = zqT.T                            # [token, feature]
        zq_real[b] = zq[:, :D]
        zq_imag[b] = zq[:, D:]
        eneg_total += r["eneg"].sum(dtype=np.float64)
    energy = np.float32(-eneg_total / (B * S * N))
    return zq_real, zq_imag, energy


def kernel(z_real, z_imag, prev_sym_dist, codebook, adjacency_energy):
    if "nc" not in _CACHE:
        _CACHE["nc"] = build_nc()
    nc = _CACHE["nc"]
    in_maps = prep_inputs(z_real, z_imag, prev_sym_dist, codebook,
                          adjacency_energy)
    res = run_bass_kernel_spmd(nc, in_maps, list(range(NCORES)))
    return postprocess(res.results)
